# revision 45
# baseline (speedup 1.0000x reference)
"""Trainium2 Bass kernel for nn_NumAttention (sparse_attention).

Reference computation (per batch b, head i):
    k     = blockmix(x_cat, softmax(W_K)[i])            # [P, DH]
    xq    = blockmix(x_cat, softmax(W_Q)[i])            # [P, DH]
    q     = xq @ softmax(W_pred)[i]                     # [P, DH]
    v     = x_num @ softmax(W_V)[i]                     # [P]
    z[qp] = sum_{p<=qp} v[p] * (k[p] . q[qp])           # causal, no softmax

Attention here is softmax-free with scalar values, so it is *linear*:
z[qp] = xq[qp] . S[qp] with S = cumsum_p(v[p] * ktilde[p,:]) where
ktilde = k @ pp^T folds the W_pred mix into the k side.  The O(P^2)
score matrix is never materialized.

This version runs as ONE fused, software-pipelined stream over the 8
chunk pairs (2x128 positions each): per pair, 8 mix matmuls produce
ktilde|xq, then three pass-2 matmuls produce S directly:
    S_pair = triT_incl @ [vk_L | vk_R]          (intra-chunk cumsum)
           + onesmat @ vk_L -> right half       (left chunk's colsum)
           + selr @ [s127 | s127] (stride-0)    (inter-pair carry)
The carry needs no prefix machinery: the inclusive cumsum's LAST ROW
already is the running total, so D_{j+1} = S_pair[127, right-half] --
staged by one tiny row-copy into s127 and broadcast to all 128 rows by
the selector matmul (selr has ones in row 127 only).  Pass-2 of pair
j-1 is emitted between the two mix chunks of pair j so the PE never
stalls on the DVE/ACT drains, and the first two pairs run kc-major so
the PE rides the DMA arrival staircase without resetting the HAM busy
window (warm 2.4 GHz clock from ~10us on).  The last pair drains as
two half-width PSUM groups reading PSUM directly, and all but its 4KB
of output ships one pair early, to cut the tail latency.

Sharding: 8 cores = 4 batches x 2 head-groups (4 heads each).  Host
ships x_cat[b] pre-transposed feature-major bf16 in per-chunk slices
(1KB descriptors) and w in per-kc slices, issued on the two HWDGE rings
in consumption order so the first matmul gates on ~286KB, not 768KB.
"""

import numpy as np
import ml_dtypes

import concourse.bacc as bacc
import concourse.mybir as mybir
import concourse.tile as tile
from concourse.bass_utils import run_bass_kernel_spmd

B, P, DC, DN, H, DH = 4, 2048, 512, 64, 8, 64
NV = DC // DH
CH = 128          # positions per chunk
NCH = P // CH     # 16 chunks
NPR = NCH // 2    # 8 chunk pairs
HPC = 4           # heads per core
FH = HPC * DH     # 256 = stacked-head free width
FH2 = 2 * FH      # 512 = ktilde|q width
NCORES = 8
KC = DC // CH     # 4 feature K-chunks
NWARM = 26        # PE warm-up dummy matmuls (N=128 each)

_BF16 = ml_dtypes.bfloat16

_cache = {}


def _softmax(x, axis=-1):
    e = np.exp(x - x.max(axis=axis, keepdims=True))
    return e / e.sum(axis=axis, keepdims=True)


def _build_program():
    nc = bacc.Bacc()
    f32 = mybir.dt.float32
    bf16 = mybir.dt.bfloat16
    mult = mybir.AluOpType.mult
    add = mybir.AluOpType.add

    w_d = nc.dram_tensor("w", [CH, KC, FH2], bf16, kind="ExternalInput")
    # per-chunk feature-major slices: [c, p128, kc, 128] -> 1KB runs
    xct_d = nc.dram_tensor("xct", [NCH, CH, KC, CH], bf16, kind="ExternalInput")
    # host-computed v in pos-chunk-major layout [p, (chunk, head)]
    v_d = nc.dram_tensor("v", [CH, NCH * HPC], f32, kind="ExternalInput")
    trit_d = nc.dram_tensor("trit", [CH, CH], bf16, kind="ExternalInput")
    selr_d = nc.dram_tensor("selr", [CH, CH], bf16, kind="ExternalInput")
    z_d = nc.dram_tensor("z", [CH, NCH * HPC], f32, kind="ExternalOutput")

    with tile.TileContext(nc) as tc:
        with (
            tc.tile_pool(name="persist", bufs=1) as pers,
            tc.tile_pool(name="work", bufs=3) as work,
            tc.tile_pool(name="mixp", bufs=4, space="PSUM") as mixp,
            tc.tile_pool(name="sp", bufs=2, space="PSUM") as sp,
            tc.tile_pool(name="psmall", bufs=1, space="PSUM") as psmall,
        ):
            w_sb = pers.tile([CH, KC, FH2], bf16, tag="w_sb")
            xcT = pers.tile([CH, NCH, KC, CH], bf16, tag="xcT")
            v_sb = pers.tile([CH, NCH * HPC], f32, tag="v_sb")
            trit_sb = pers.tile([CH, CH], bf16, tag="trit_sb")
            selr_sb = pers.tile([CH, CH], bf16, tag="selr_sb")
            onesm = pers.tile([CH, CH], bf16, tag="onesm")
            vk_sb = pers.tile([CH, NCH, FH], bf16, tag="vk_sb")
            q_sb = pers.tile([CH, NCH, FH], bf16, tag="q_sb")
            s127 = pers.tile([CH, FH], bf16, tag="s127")
            z_sb = pers.tile([CH, NCH * HPC], f32, tag="z_sb")
            dumw = pers.tile([CH, CH], bf16, tag="dumw")

            # ---- PE warm-up: dummy matmuls on a memset tile release the HAM
            # clock throttle while the input DMAs are still in flight
            nc.gpsimd.memset(dumw[:], 0.0)
            nc.gpsimd.memset(onesm[:], 1.0)
            # rows != 127 must be zero (selr multiplies them by 0, but
            # uninitialized SBUF could hold Inf patterns -> 0*Inf = NaN)
            nc.gpsimd.memset(s127[:], 0.0)
            psum_dum = psmall.tile([CH, FH2], f32, tag="psum_dum")
            for i in range(NWARM):
                nc.tensor.matmul(
                    psum_dum[:, 0:CH], dumw[:], dumw[:], start=True, stop=True
                )

            # ---- loads.  Each HWDGE ring drains FIFO; the rings share the
            # SDMA engines round-robin, so issue in consumption order for
            # the kc-major head: w[kc0] + chunks 0-3 first, then w[kc1..3].
            nc.scalar.dma_start(out=w_sb[:, 0, :], in_=w_d[:, 0, :])
            nc.sync.dma_start(out=xcT[:, 0], in_=xct_d[0])
            nc.scalar.dma_start(out=xcT[:, 1], in_=xct_d[1])
            nc.sync.dma_start(out=xcT[:, 2], in_=xct_d[2])
            nc.scalar.dma_start(out=xcT[:, 3], in_=xct_d[3])
            nc.sync.dma_start(out=w_sb[:, 1, :], in_=w_d[:, 1, :])
            nc.scalar.dma_start(out=w_sb[:, 2, :], in_=w_d[:, 2, :])
            nc.sync.dma_start(out=w_sb[:, 3, :], in_=w_d[:, 3, :])
            nc.scalar.dma_start(out=xcT[:, 4], in_=xct_d[4])
            nc.sync.dma_start(out=xcT[:, 5], in_=xct_d[5])
            nc.scalar.dma_start(out=v_sb[:], in_=v_d[:])
            nc.sync.dma_start(out=trit_sb[:], in_=trit_d[:])
            nc.scalar.dma_start(out=xcT[:, 6], in_=xct_d[6])
            nc.sync.dma_start(out=selr_sb[:], in_=selr_d[:])
            for c in range(7, NCH):
                ring = nc.sync if c % 2 else nc.scalar
                ring.dma_start(out=xcT[:, c], in_=xct_d[c])

            def mix_mm(c, psum_mix, kc):
                nc.tensor.matmul(
                    psum_mix[:],
                    xcT[:, c, kc, :],
                    w_sb[:, kc, :],
                    start=(kc == 0),
                    stop=(kc == KC - 1),
                )

            def mix_drain(c, psum_mix):
                # vk[p, i, h] = ktilde[p, i, h] * v[p, i]
                nc.vector.tensor_tensor(
                    out=vk_sb[:, c, :].rearrange("p (i h) -> p i h", h=DH),
                    in0=psum_mix[:, 0:FH].rearrange("p (i h) -> p i h", h=DH),
                    in1=v_sb[:, c * HPC : (c + 1) * HPC]
                    .unsqueeze(2)
                    .broadcast_to([CH, HPC, DH]),
                    op=mult,
                )
                nc.scalar.copy(q_sb[:, c, :], psum_mix[:, FH:FH2])

            def mix_chunk(c):
                psum_mix = mixp.tile([CH, FH2], f32, tag="psum_mix", name="pm")
                for kc in range(KC):
                    mix_mm(c, psum_mix, kc)
                mix_drain(c, psum_mix)

            def pass2_mm(j):
                # S = triT@[vk_L|vk_R] (+ colsum(vk_L) into right half)
                # (+ carry D = s127[127, :] broadcast to all rows)
                psum_S = sp.tile([CH, FH2], f32, tag="psum_S")
                nc.tensor.matmul(
                    psum_S[:],
                    trit_sb[:],
                    vk_sb[:, 2 * j : 2 * j + 2, :].rearrange("p c f -> p (c f)"),
                    start=True,
                    stop=False,
                )
                nc.tensor.matmul(
                    psum_S[:, FH:FH2],
                    onesm[:],
                    vk_sb[:, 2 * j, :],
                    start=False,
                    stop=(j == 0),
                )
                if j > 0:
                    # one N=512 carry matmul: rhs broadcasts s127 to both
                    # halves via a stride-0 middle dim
                    nc.tensor.matmul(
                        psum_S[:].rearrange("p (c f) -> p c f", f=FH),
                        selr_sb[:],
                        s127[:].unsqueeze(1).broadcast_to([CH, 2, FH]),
                        start=False, stop=True,
                    )
                return psum_S

            def pass2_last(j):
                # tail-latency special case: two half-width PSUM groups so
                # the left chunk drains while the right chunk's matmuls run;
                # mult/reduce read PSUM directly (no ACT staging), with the
                # left reduce on gpsimd to overlap the DVE chain
                for c in range(2):
                    ps = sp.tile([CH, FH2], f32, tag="psum_S", name=f"psl{c}")
                    nc.tensor.matmul(
                        ps[:, 0:FH],
                        trit_sb[:],
                        vk_sb[:, 2 * j + c, :],
                        start=True,
                        stop=False,
                    )
                    if c == 1:
                        nc.tensor.matmul(
                            ps[:, 0:FH],
                            onesm[:],
                            vk_sb[:, 2 * j, :],
                            start=False,
                            stop=False,
                        )
                    nc.tensor.matmul(
                        ps[:, 0:FH], selr_sb[:], s127[:],
                        start=False, stop=True,
                    )
                    prod = work.tile([CH, FH], bf16, tag="prodl", name=f"prl{c}")
                    nc.vector.tensor_tensor(
                        out=prod[:],
                        in0=q_sb[:, 2 * j + c, :],
                        in1=ps[:, 0:FH],
                        op=mult,
                    )
                    nc.vector.tensor_reduce(
                        out=z_sb[:, (2 * j + c) * HPC : (2 * j + c + 1) * HPC],
                        in_=prod[:].rearrange("p (i h) -> p i h", h=DH),
                        axis=mybir.AxisListType.X,
                        op=add,
                    )

            def pass2_drain(j, psum_S):
                # stage next pair's carry first: a ~100ns row-127 copy keeps
                # the pair-to-pair carry link off the fat s_sb copy
                if j < NPR - 1:
                    # partition ranges must be 32-aligned: copy rows 96-127
                    # (selr zeroes all but row 127 anyway)
                    nc.scalar.copy(
                        s127[CH - 32 : CH, :], psum_S[CH - 32 : CH, FH:FH2]
                    )
                s_sb = work.tile([CH, FH2], bf16, tag="s_sb")
                nc.scalar.copy(s_sb[:], psum_S[:])
                prod = work.tile([CH, FH2], bf16, tag="prod")
                nc.vector.tensor_tensor(
                    out=prod[:],
                    in0=q_sb[:, 2 * j : 2 * j + 2, :].rearrange("p c f -> p (c f)"),
                    in1=s_sb[:],
                    op=mult,
                )
                nc.vector.tensor_reduce(
                    out=z_sb[:, 2 * j * HPC : (2 * j + 2) * HPC],
                    in_=prod[:].rearrange("p (ci h) -> p ci h", h=DH),
                    axis=mybir.AxisListType.X,
                    op=add,
                )

            # ---- head: chunks 0-3 in staggered kc-major groups, each group
            # adding only ONE new DMA dependency, matched to the ~2
            # transfers/us early delivery rate (a stall would reset the HAM
            # busy window and delay the warm clock by several us)
            head_psums = [
                mixp.tile([CH, FH2], f32, tag="psum_mix", name=f"pmh{c}")
                for c in range(4)
            ]
            for kc in range(KC):
                for c in (0, 1):
                    mix_mm(c, head_psums[c], kc)
                    if kc == KC - 1:
                        mix_drain(c, head_psums[c])
                for c in (2, 3):
                    mix_mm(c, head_psums[c], kc)
                    if kc == KC - 1:
                        mix_drain(c, head_psums[c])

            # ---- fused stream, pass-2 pipelined one pair behind the mix
            for j in range(1, NPR):
                if 2 * j + 2 < NCH:
                    mix_chunk(2 * j + 2)
                psum_S = pass2_mm(j - 1)
                if 2 * j + 3 < NCH:
                    mix_chunk(2 * j + 3)
                pass2_drain(j - 1, psum_S)
                if j == NPR - 1:
                    # ship all but the last pair's output early; only the
                    # final 4KB slice waits on the tail drain
                    nc.sync.dma_start(
                        out=z_d[:, 0 : (NPR - 1) * 2 * HPC],
                        in_=z_sb[:, 0 : (NPR - 1) * 2 * HPC],
                    )
            pass2_last(NPR - 1)

            # scalar ring: idle at the tail, while sync still drains bulk-z
            nc.scalar.dma_start(
                out=z_d[:, (NPR - 1) * 2 * HPC :],
                in_=z_sb[:, (NPR - 1) * 2 * HPC :],
            )

    nc.finalize()
    return nc


def _host_inputs(x_cat, x_num, W_K, W_Q, W_pred, W_V):
    """Per-core input maps. Core c = batch (c//2), head-group (c%2)."""
    pk = _softmax(W_K.astype(np.float64)).astype(np.float32)
    pq = _softmax(W_Q.astype(np.float64)).astype(np.float32)
    pp = _softmax(W_pred.astype(np.float64)).astype(np.float32)
    pv = _softmax(W_V.astype(np.float64)).astype(np.float32)

    trit = np.triu(np.ones((CH, CH), np.float32))
    selr = np.zeros((CH, CH), np.float32)
    selr[CH - 1, :] = 1.0
    eye = np.eye(DH, dtype=np.float32)
    v_full = np.einsum("bpd,id->bpi", x_num, pv)  # [B, P, H] fp32, host-side

    in_maps = []
    for core in range(NCORES):
        b, hg = core // 2, core % 2
        heads = range(hg * HPC, (hg + 1) * HPC)
        W = np.zeros((DC, FH2), np.float32)
        for j, i in enumerate(heads):
            # ktilde cols: W[(v,g), j*64+h] = pk[i,v] * pp[i,h,g]
            W[:, j * DH : (j + 1) * DH] = (
                pk[i][:, None, None] * pp[i].T[None, :, :]
            ).reshape(DC, DH)
            # xq cols: W[(v,h), FH + j*64+h'] = pq[i,v] * delta(h,h')
            W[:, FH + j * DH : FH + (j + 1) * DH] = np.kron(pq[i][:, None], eye)
        # per-chunk feature-major slices, [kc, 128] contiguous per partition
        xq16 = x_cat[b].T.reshape(KC, CH, NCH, CH).transpose(2, 1, 0, 3)
        wq = W.reshape(KC, CH, FH2).transpose(1, 0, 2)
        # v in device layout [p, (chunk, head)]
        v_core = v_full[b][:, hg * HPC : (hg + 1) * HPC]  # [P, HPC]
        v_dev = np.ascontiguousarray(
            v_core.reshape(NCH, CH, HPC).transpose(1, 0, 2).reshape(CH, NCH * HPC)
        )
        in_maps.append(
            {
                "xct": np.ascontiguousarray(xq16).astype(_BF16),
                "w": np.ascontiguousarray(wq).astype(_BF16),
                "v": v_dev,
                "trit": trit.astype(_BF16),
                "selr": selr.astype(_BF16),
            }
        )
    return in_maps


def _run(inputs, **spmd_kwargs):
    if "nc" not in _cache:
        _cache["nc"] = _build_program()
    nc = _cache["nc"]

    in_maps = _host_inputs(**inputs)
    res = run_bass_kernel_spmd(nc, in_maps, list(range(NCORES)), **spmd_kwargs)

    out = np.zeros((B, P, H), np.float32)
    for core in range(NCORES):
        b, hg = core // 2, core % 2
        z = res.results[core]["z"]  # [128, NCH*HPC]
        z = z.reshape(CH, NCH, HPC).transpose(1, 0, 2).reshape(P, HPC)
        out[b, :, hg * HPC : (hg + 1) * HPC] = z
    return out, res


def kernel(x_cat, x_num, W_K, W_Q, W_pred, W_V):
    out, _ = _run(
        dict(x_cat=x_cat, x_num=x_num, W_K=W_K, W_Q=W_Q, W_pred=W_pred, W_V=W_V)
    )
    return out


# revision 47
# speedup vs baseline: 1.0127x; 1.0127x over previous
"""Trainium2 Bass kernel for nn_NumAttention (sparse_attention).

Reference computation (per batch b, head i):
    k     = blockmix(x_cat, softmax(W_K)[i])            # [P, DH]
    xq    = blockmix(x_cat, softmax(W_Q)[i])            # [P, DH]
    q     = xq @ softmax(W_pred)[i]                     # [P, DH]
    v     = x_num @ softmax(W_V)[i]                     # [P]
    z[qp] = sum_{p<=qp} v[p] * (k[p] . q[qp])           # causal, no softmax

Attention here is softmax-free with scalar values, so it is *linear*:
z[qp] = xq[qp] . S[qp] with S = cumsum_p(v[p] * ktilde[p,:]) where
ktilde = k @ pp^T folds the W_pred mix into the k side.  The O(P^2)
score matrix is never materialized.

This version runs as ONE fused, software-pipelined stream over the 8
chunk pairs (2x128 positions each): per pair, 8 mix matmuls produce
ktilde|xq, then three pass-2 matmuls produce S directly:
    S_pair = triT_incl @ [vk_L | vk_R]          (intra-chunk cumsum)
           + onesmat @ vk_L -> right half       (left chunk's colsum)
           + selr @ [s127 | s127] (stride-0)    (inter-pair carry)
The carry needs no prefix machinery: the inclusive cumsum's LAST ROW
already is the running total, so D_{j+1} = S_pair[127, right-half] --
staged by one tiny row-copy into s127 and broadcast to all 128 rows by
the selector matmul (selr has ones in row 127 only).  Pass-2 of pair
j-1 is emitted between the two mix chunks of pair j so the PE never
stalls on the DVE/ACT drains, and the first two pairs run kc-major so
the PE rides the DMA arrival staircase without resetting the HAM busy
window (warm 2.4 GHz clock from ~10us on).  The last pair drains as
two half-width PSUM groups reading PSUM directly, and all but its 4KB
of output ships one pair early, to cut the tail latency.

Sharding: 8 cores = 4 batches x 2 head-groups (4 heads each).  Host
ships x_cat[b] pre-transposed feature-major bf16 in per-chunk slices
(1KB descriptors) and w in per-kc slices, issued on the two HWDGE rings
in consumption order so the first matmul gates on ~286KB, not 768KB.
"""

import numpy as np
import ml_dtypes

import concourse.bacc as bacc
import concourse.mybir as mybir
import concourse.tile as tile
from concourse.bass_utils import run_bass_kernel_spmd

B, P, DC, DN, H, DH = 4, 2048, 512, 64, 8, 64
NV = DC // DH
CH = 128          # positions per chunk
NCH = P // CH     # 16 chunks
NPR = NCH // 2    # 8 chunk pairs
HPC = 4           # heads per core
FH = HPC * DH     # 256 = stacked-head free width
FH2 = 2 * FH      # 512 = ktilde|q width
NCORES = 8
KC = DC // CH     # 4 feature K-chunks
NWARM = 26        # PE warm-up dummy matmuls (N=128 each)

_BF16 = ml_dtypes.bfloat16

_cache = {}


def _softmax(x, axis=-1):
    e = np.exp(x - x.max(axis=axis, keepdims=True))
    return e / e.sum(axis=axis, keepdims=True)


def _build_program():
    nc = bacc.Bacc()
    f32 = mybir.dt.float32
    bf16 = mybir.dt.bfloat16
    mult = mybir.AluOpType.mult
    add = mybir.AluOpType.add

    w_d = nc.dram_tensor("w", [CH, KC, FH2], bf16, kind="ExternalInput")
    # per-chunk feature-major slices: [c, p128, kc, 128] -> 1KB runs
    xct_d = nc.dram_tensor("xct", [NCH, CH, KC, CH], bf16, kind="ExternalInput")
    # host-computed v in pos-chunk-major layout [p, (chunk, head)]
    v_d = nc.dram_tensor("v", [CH, NCH * HPC], f32, kind="ExternalInput")
    trit_d = nc.dram_tensor("trit", [CH, CH], bf16, kind="ExternalInput")
    selr_d = nc.dram_tensor("selr", [CH, CH], bf16, kind="ExternalInput")
    z_d = nc.dram_tensor("z", [CH, NCH * HPC], f32, kind="ExternalOutput")

    with tile.TileContext(nc) as tc:
        with (
            tc.tile_pool(name="persist", bufs=1) as pers,
            tc.tile_pool(name="work", bufs=3) as work,
            tc.tile_pool(name="mixp", bufs=4, space="PSUM") as mixp,
            tc.tile_pool(name="sp", bufs=2, space="PSUM") as sp,
            tc.tile_pool(name="psmall", bufs=1, space="PSUM") as psmall,
        ):
            w_sb = pers.tile([CH, KC, FH2], bf16, tag="w_sb")
            xcT = pers.tile([CH, NCH, KC, CH], bf16, tag="xcT")
            v_sb = pers.tile([CH, NCH * HPC], f32, tag="v_sb")
            trit_sb = pers.tile([CH, CH], bf16, tag="trit_sb")
            selr_sb = pers.tile([CH, CH], bf16, tag="selr_sb")
            onesm = pers.tile([CH, CH], bf16, tag="onesm")
            vk_sb = pers.tile([CH, NCH, FH], bf16, tag="vk_sb")
            q_sb = pers.tile([CH, NCH, FH], bf16, tag="q_sb")
            s127 = pers.tile([CH, FH], bf16, tag="s127")
            z_sb = pers.tile([CH, NCH * HPC], f32, tag="z_sb")
            dumw = pers.tile([CH, CH], bf16, tag="dumw")

            # ---- PE warm-up: dummy matmuls on a memset tile release the HAM
            # clock throttle while the input DMAs are still in flight
            nc.gpsimd.memset(dumw[:], 0.0)
            nc.gpsimd.memset(onesm[:], 1.0)
            # rows != 127 must be zero (selr multiplies them by 0, but
            # uninitialized SBUF could hold Inf patterns -> 0*Inf = NaN)
            nc.gpsimd.memset(s127[:], 0.0)
            psum_dum = psmall.tile([CH, FH2], f32, tag="psum_dum")
            for i in range(NWARM):
                nc.tensor.matmul(
                    psum_dum[:, 0:CH], dumw[:], dumw[:], start=True, stop=True
                )

            # ---- loads.  Each HWDGE ring drains FIFO; the rings share the
            # SDMA engines round-robin, so issue in consumption order for
            # the kc-major head: w[kc0] + chunks 0-3 first, then w[kc1..3].
            nc.scalar.dma_start(out=w_sb[:, 0, :], in_=w_d[:, 0, :])
            nc.sync.dma_start(out=xcT[:, 0], in_=xct_d[0])
            nc.scalar.dma_start(out=xcT[:, 1], in_=xct_d[1])
            nc.sync.dma_start(out=xcT[:, 2], in_=xct_d[2])
            nc.scalar.dma_start(out=xcT[:, 3], in_=xct_d[3])
            nc.sync.dma_start(out=w_sb[:, 1, :], in_=w_d[:, 1, :])
            nc.scalar.dma_start(out=w_sb[:, 2, :], in_=w_d[:, 2, :])
            nc.sync.dma_start(out=w_sb[:, 3, :], in_=w_d[:, 3, :])
            nc.scalar.dma_start(out=xcT[:, 4], in_=xct_d[4])
            nc.sync.dma_start(out=xcT[:, 5], in_=xct_d[5])
            nc.scalar.dma_start(out=v_sb[:], in_=v_d[:])
            nc.sync.dma_start(out=trit_sb[:], in_=trit_d[:])
            nc.scalar.dma_start(out=xcT[:, 6], in_=xct_d[6])
            nc.sync.dma_start(out=selr_sb[:], in_=selr_d[:])
            for c in range(7, NCH):
                ring = nc.sync if c % 2 else nc.scalar
                ring.dma_start(out=xcT[:, c], in_=xct_d[c])

            def mix_mm(c, psum_mix, kc):
                nc.tensor.matmul(
                    psum_mix[:],
                    xcT[:, c, kc, :],
                    w_sb[:, kc, :],
                    start=(kc == 0),
                    stop=(kc == KC - 1),
                )

            def mix_drain(c, psum_mix):
                # vk[p, i, h] = ktilde[p, i, h] * v[p, i]
                nc.vector.tensor_tensor(
                    out=vk_sb[:, c, :].rearrange("p (i h) -> p i h", h=DH),
                    in0=psum_mix[:, 0:FH].rearrange("p (i h) -> p i h", h=DH),
                    in1=v_sb[:, c * HPC : (c + 1) * HPC]
                    .unsqueeze(2)
                    .broadcast_to([CH, HPC, DH]),
                    op=mult,
                )
                nc.scalar.copy(q_sb[:, c, :], psum_mix[:, FH:FH2])

            def mix_chunk(c):
                psum_mix = mixp.tile([CH, FH2], f32, tag="psum_mix", name="pm")
                for kc in range(KC):
                    mix_mm(c, psum_mix, kc)
                mix_drain(c, psum_mix)

            def pass2_mm(j):
                # S = triT@[vk_L|vk_R] (+ colsum(vk_L) into right half)
                # (+ carry D = s127[127, :] broadcast to all rows)
                psum_S = sp.tile([CH, FH2], f32, tag="psum_S")
                nc.tensor.matmul(
                    psum_S[:],
                    trit_sb[:],
                    vk_sb[:, 2 * j : 2 * j + 2, :].rearrange("p c f -> p (c f)"),
                    start=True,
                    stop=False,
                )
                nc.tensor.matmul(
                    psum_S[:, FH:FH2],
                    onesm[:],
                    vk_sb[:, 2 * j, :],
                    start=False,
                    stop=(j == 0),
                )
                if j > 0:
                    # one N=512 carry matmul: rhs broadcasts s127 to both
                    # halves via a stride-0 middle dim
                    nc.tensor.matmul(
                        psum_S[:].rearrange("p (c f) -> p c f", f=FH),
                        selr_sb[:],
                        s127[:].unsqueeze(1).broadcast_to([CH, 2, FH]),
                        start=False, stop=True,
                    )
                return psum_S

            def pass2_last(j):
                # tail-latency special case: two half-width PSUM groups so
                # the left chunk drains while the right chunk's matmuls run;
                # mult/reduce read PSUM directly (no ACT staging), with the
                # left reduce on gpsimd to overlap the DVE chain
                for c in range(2):
                    ps = sp.tile([CH, FH2], f32, tag="psum_S", name=f"psl{c}")
                    nc.tensor.matmul(
                        ps[:, 0:FH],
                        trit_sb[:],
                        vk_sb[:, 2 * j + c, :],
                        start=True,
                        stop=False,
                    )
                    if c == 1:
                        nc.tensor.matmul(
                            ps[:, 0:FH],
                            onesm[:],
                            vk_sb[:, 2 * j, :],
                            start=False,
                            stop=False,
                        )
                    nc.tensor.matmul(
                        ps[:, 0:FH], selr_sb[:], s127[:],
                        start=False, stop=True,
                    )
                    prod = work.tile([CH, FH], bf16, tag="prodl", name=f"prl{c}")
                    nc.vector.tensor_tensor(
                        out=prod[:],
                        in0=q_sb[:, 2 * j + c, :],
                        in1=ps[:, 0:FH],
                        op=mult,
                    )
                    nc.vector.tensor_reduce(
                        out=z_sb[:, (2 * j + c) * HPC : (2 * j + c + 1) * HPC],
                        in_=prod[:].rearrange("p (i h) -> p i h", h=DH),
                        axis=mybir.AxisListType.X,
                        op=add,
                    )

            def pass2_drain(j, psum_S):
                # stage next pair's carry first: a ~100ns row-127 copy keeps
                # the pair-to-pair carry link off the fat s_sb copy
                if j < NPR - 1:
                    # partition ranges must be 32-aligned: copy rows 96-127
                    # (selr zeroes all but row 127 anyway)
                    nc.scalar.copy(
                        s127[CH - 32 : CH, :], psum_S[CH - 32 : CH, FH:FH2]
                    )
                s_sb = work.tile([CH, FH2], bf16, tag="s_sb")
                nc.scalar.copy(s_sb[:], psum_S[:])
                prod = work.tile([CH, FH2], bf16, tag="prod")
                nc.vector.tensor_tensor(
                    out=prod[:],
                    in0=q_sb[:, 2 * j : 2 * j + 2, :].rearrange("p c f -> p (c f)"),
                    in1=s_sb[:],
                    op=mult,
                )
                nc.vector.tensor_reduce(
                    out=z_sb[:, 2 * j * HPC : (2 * j + 2) * HPC],
                    in_=prod[:].rearrange("p (ci h) -> p ci h", h=DH),
                    axis=mybir.AxisListType.X,
                    op=add,
                )

            # ---- head: chunks 0-3 in staggered kc-major groups, each group
            # adding only ONE new DMA dependency, matched to the ~2
            # transfers/us early delivery rate (a stall would reset the HAM
            # busy window and delay the warm clock by several us)
            head_psums = [
                mixp.tile([CH, FH2], f32, tag="psum_mix", name=f"pmh{c}")
                for c in range(4)
            ]
            for kc in range(KC):
                for c in (0, 1):
                    mix_mm(c, head_psums[c], kc)
                    if kc == KC - 1:
                        mix_drain(c, head_psums[c])
                for c in (2, 3):
                    mix_mm(c, head_psums[c], kc)
                    if kc == KC - 1:
                        mix_drain(c, head_psums[c])

            # ---- fused stream, pass-2 pipelined one pair behind the mix
            for j in range(1, NPR):
                if 2 * j + 2 < NCH:
                    mix_chunk(2 * j + 2)
                psum_S = pass2_mm(j - 1)
                if 2 * j + 3 < NCH:
                    mix_chunk(2 * j + 3)
                pass2_drain(j - 1, psum_S)
                if j == NPR - 1:
                    # ship all but the last pair's output early; only the
                    # final 4KB slice waits on the tail drain
                    nc.sync.dma_start(
                        out=z_d[:, 0 : (NPR - 1) * 2 * HPC],
                        in_=z_sb[:, 0 : (NPR - 1) * 2 * HPC],
                    )
            pass2_last(NPR - 1)

            # scalar ring: idle at the tail, while sync still drains bulk-z
            nc.scalar.dma_start(
                out=z_d[:, (NPR - 1) * 2 * HPC :],
                in_=z_sb[:, (NPR - 1) * 2 * HPC :],
            )

    nc.finalize()
    return nc


def _host_inputs(x_cat, x_num, W_K, W_Q, W_pred, W_V):
    """Per-core input maps. Core c = batch (c//2), head-group (c%2)."""
    pk = _softmax(W_K.astype(np.float64)).astype(np.float32)
    pq = _softmax(W_Q.astype(np.float64)).astype(np.float32)
    pp = _softmax(W_pred.astype(np.float64)).astype(np.float32)
    pv = _softmax(W_V.astype(np.float64)).astype(np.float32)

    trit = np.triu(np.ones((CH, CH), np.float32))
    selr = np.zeros((CH, CH), np.float32)
    selr[CH - 1, :] = 1.0
    eye = np.eye(DH, dtype=np.float32)
    v_full = np.einsum("bpd,id->bpi", x_num, pv)  # [B, P, H] fp32, host-side

    in_maps = []
    for core in range(NCORES):
        b, hg = core // 2, core % 2
        heads = range(hg * HPC, (hg + 1) * HPC)
        W = np.zeros((DC, FH2), np.float32)
        for j, i in enumerate(heads):
            # ktilde cols: W[(v,g), j*64+h] = pk[i,v] * pp[i,h,g]
            W[:, j * DH : (j + 1) * DH] = (
                pk[i][:, None, None] * pp[i].T[None, :, :]
            ).reshape(DC, DH)
            # xq cols: W[(v,h), FH + j*64+h'] = pq[i,v] * delta(h,h')
            W[:, FH + j * DH : FH + (j + 1) * DH] = np.kron(pq[i][:, None], eye)
        # per-chunk feature-major slices, [kc, 128] contiguous per partition
        xq16 = x_cat[b].T.reshape(KC, CH, NCH, CH).transpose(2, 1, 0, 3)
        wq = W.reshape(KC, CH, FH2).transpose(1, 0, 2)
        # v in device layout [p, (chunk, head)]
        v_core = v_full[b][:, hg * HPC : (hg + 1) * HPC]  # [P, HPC]
        v_dev = np.ascontiguousarray(
            v_core.reshape(NCH, CH, HPC).transpose(1, 0, 2).reshape(CH, NCH * HPC)
        )
        in_maps.append(
            {
                "xct": np.ascontiguousarray(xq16).astype(_BF16),
                "w": np.ascontiguousarray(wq).astype(_BF16),
                "v": v_dev,
                "trit": trit.astype(_BF16),
                "selr": selr.astype(_BF16),
            }
        )
    return in_maps


def _run(inputs, **spmd_kwargs):
    if "nc" not in _cache:
        _cache["nc"] = _build_program()
    nc = _cache["nc"]

    in_maps = _host_inputs(**inputs)
    res = run_bass_kernel_spmd(nc, in_maps, list(range(NCORES)), **spmd_kwargs)

    out = np.zeros((B, P, H), np.float32)
    for core in range(NCORES):
        b, hg = core // 2, core % 2
        z = res.results[core]["z"]  # [128, NCH*HPC]
        z = z.reshape(CH, NCH, HPC).transpose(1, 0, 2).reshape(P, HPC)
        out[b, :, hg * HPC : (hg + 1) * HPC] = z
    return out, res


def kernel(x_cat, x_num, W_K, W_Q, W_pred, W_V):
    out, _ = _run(
        dict(x_cat=x_cat, x_num=x_num, W_K=W_K, W_Q=W_Q, W_pred=W_pred, W_V=W_V)
    )
    return out
